# revision 18
# baseline (speedup 1.0000x reference)
"""Trainium2 Bass kernel for nn_CrossAttention_28183575396415.

The reference block-mask gives every query exactly one key (kv = q_idx // 3),
so the softmax weight is identically 1 and the q/k projections, RMSNorm and
RoPE are dead code.  The module reduces to

    out[b, t] = x_kv[b, t // 3] @ Wv.T @ Wproj.T
              = x_kv[b, t // 3] @ WfT          with WfT = Wv.T @ Wproj.T

Strategy (8 NeuronCores, SPMD):
  - Host folds the two projection matrices into WfT (computed in float64)
    - constant folding of adjacent linear layers.
  - The 4*2048 = 8192 kv rows are row-sharded 8 ways (1024 rows/core).
    Each core's shard is pre-transposed on host so every device DMA is a
    natural contiguous load; the shard and the weight are concatenated into
    one [1024(k), 2048] bf16 input:
        xw[:, :1024]  = x_shard.T   (k on partitions = contraction dim)
        xw[:, 1024:]  = WfT
    The device streams cols 0:1536 (x + first W half - everything pass 0
    needs) as five k-ordered DMAs, and the pass-1-only second W half as one
    trailing DMA, so pass 0's k sems arrive ~25% sooner.
  - Device: z = xT.T @ WfT, K accumulated in PSUM over 8 k-tiles.
    Column half 0 runs k-major (overlapping the input stream), column half 1
    runs m-major so finished row tiles retire one at a time and the output
    DMA stream stays busy until the end instead of bursting at the tail.
  - Each z tile is written to HBM with a single DMA whose source AP repeats
    the tile 3x (stride-0 middle dim) - the t//3 replication - giving this
    core's contiguous [3072, 1024] slice of the flattened output in bf16.
  - Host unshard = concatenate the 8 slices and upcast to float32.
"""

import json
import os

import numpy as np

import concourse.bass as bass
import concourse.mybir as mybir
from bass_rust import AP
from concourse.tile import TileContext
from concourse.vector_clock import ScopedClock
from concourse.bass_utils import run_bass_kernel_spmd

P = 128          # partitions
C = 1024         # model dim
K_T = C // P     # k tiles
M_T = C // P     # row tiles per core shard
N = 512          # matmul free dim (one PSUM bank of fp32)
L = 3            # replication factor (Tq // Tkv)
ROWS_PER_CORE = 1024
N_CORES = 8

# compute dtype: "bf16" (half the input DMA), "f32r"/"f32" for debugging
COMPUTE_DT = os.environ.get("KERNEL_COMPUTE_DT", "bf16")
# output dtype on device: "bf16" (host upcasts) or "f32"
OUT_DT = os.environ.get("KERNEL_OUT_DT", "bf16")
# "bcast": one DMA per z tile with stride-0 replication; "multi": 3 DMAs
REP_MODE = os.environ.get("KERNEL_REP_MODE", "bcast")


class SlimTailTileContext(TileContext):
    """Tile's kernel tail is drain -> barrier -> ~280 serialized per-semaphore
    clear instructions -> barrier (~8 us measured).  The clears only matter if
    the loaded NEFF executes more than once; every kernel() call here builds a
    fresh jit executable (fresh NEFF load, semaphores re-initialized), so skip
    them and the second barrier.  The drain still waits for every DMA queue,
    so outputs are complete before the program ends."""

    def _drain_and_barrier(self, tick_clock, wait_clock):
        drain_inst = self.nc.sync.drain()
        wait_clock.add_sem_waits(
            drain_inst.ins, ScopedClock({None: tick_clock.global_clock})
        )
        popped = self.nc._tile_sem_poison_stack.pop()
        assert popped is self._sem_poison


def _split_multiwaits(nc: bass.Bass) -> None:
    """This container's walrus allows only ONE sync-wait on several
    instruction formats (Drain/CTRL, Matmult's LDWEIGHTS half, ...).  Tile
    can emit more.  Post-pass the serialized BIR: for any instruction with
    >1 on_wait, hoist all but the last wait onto single-wait EventSemaphore
    carriers inserted immediately before it on the same engine (waits then
    execute in queue order - semantics unchanged)."""
    raw = bass.Bass.to_json_bytes(nc)
    j = json.loads(raw)
    for f in j["functions"]:
        for bb in f["blocks"]:
            new_insts = []
            for ins in bb["instructions"]:
                si = ins.get("sync_info")
                waits = si.get("on_wait", []) if si else []
                if len(waits) > 1:
                    for i, w in enumerate(waits[:-1]):
                        carrier = {
                            "engine": ins["engine"],
                            "ins": [],
                            "outs": [],
                            "name": f"{ins['name']}_hw{i}",
                            "opcode": "EventSemaphore",
                            "sync_info": {"on_update": [], "on_wait": [w]},
                        }
                        if "debug" in ins:
                            carrier["debug"] = ins["debug"]
                        new_insts.append(carrier)
                    si["on_wait"] = waits[-1:]
                new_insts.append(ins)
            bb["instructions"] = new_insts
    patched = json.dumps(j).encode()
    nc.to_json_bytes = lambda: patched


def _rep3_src(zh_ap):
    """Source AP reading a [P, N] SBUF tile as [P, L, N] via a stride-0
    middle dim - the DMA replicates each row L times."""
    lay = zh_ap.ap
    assert len(lay) == 2, lay
    return AP(tensor=zh_ap.tensor, offset=zh_ap.offset, ap=[lay[0], [0, L], lay[1]])


def _build(compute_dt: str, out_dt: str, rep_mode: str) -> bass.Bass:
    nc = bass.Bass("TRN2")
    in_mydt = {
        "bf16": mybir.dt.bfloat16,
        "f32r": mybir.dt.float32r,
        "f32": mybir.dt.float32,
    }[compute_dt]
    out_mydt = {"bf16": mybir.dt.bfloat16, "f32": mybir.dt.float32}[out_dt]

    W2 = ROWS_PER_CORE + C  # concatenated [x | w] free dim
    xw = nc.dram_tensor("xw", [C, W2], in_mydt, kind="ExternalInput")
    out = nc.dram_tensor(
        "out", [L * ROWS_PER_CORE, C], out_mydt, kind="ExternalOutput"
    )
    # out row (L*g + r) <- z row g
    out_rep = out.rearrange("(g r) c -> g r c", r=L)  # [1024, L, 1024]

    with SlimTailTileContext(nc) as tc:
        with (
            tc.tile_pool(name="xw", bufs=1) as xw_pool,
            tc.tile_pool(name="psum", bufs=8, space="PSUM") as psum_pool,
            tc.tile_pool(name="zout", bufs=16) as z_pool,
        ):
            # Input: pass 0 only needs the x columns and the first W half
            # (cols 0:1536 of each k-tile row block), so the k-tile DMAs
            # carry just those 3 MiB - each k's sem arrives ~25% sooner -
            # and the second W half (pass-1-only, not needed before ~25us)
            # follows as one final 1 MiB DMA.  First k-tile alone so the PE
            # starts as early as possible; 0.4-0.8 MiB transfers alternate
            # between the two HWDGE rings so k sems pace the PE's
            # cold-then-warm consumption without ever idling it (an idle
            # window would re-throttle the PE clock).
            W1 = ROWS_PER_CORE + N  # x | W-cc0 columns per k
            in_eng = [nc.sync, nc.scalar]
            groups = [[0], [1, 2], [3, 4], [5, 6], [7]]
            xwk = [None] * K_T
            for j, grp in enumerate(groups):
                n = len(grp)
                t = xw_pool.tile([P, n * W1], in_mydt, name=f"xwp{j}", tag=f"xwp{j}")
                src = xw[grp[0] * P : (grp[0] + n) * P, :W1].rearrange(
                    "(g p) m -> p g m", p=P
                )
                dst = t[:].rearrange("p (g m) -> p g m", g=n)
                in_eng[j % 2].dma_start(dst, src)
                for i, k in enumerate(grp):
                    xwk[k] = (t, i * W1)
            wc1 = xw_pool.tile([P, K_T * N], in_mydt, name="wc1", tag="wc1")
            nc.scalar.dma_start(
                wc1[:].rearrange("p (k m) -> p k m", k=K_T),
                xw[:, W1:].rearrange("(k p) m -> p k m", p=P),
            )

            out_eng = [nc.sync, nc.scalar]
            n_trig = [0]

            def store(zh, m, cc, lo=0, hi=N, eng=None):
                dst = out_rep[m * P : (m + 1) * P, :, cc * N + lo : cc * N + hi]
                src = zh[:, lo:hi]
                if eng is None:
                    eng = out_eng[n_trig[0] % 2]
                    n_trig[0] += 1
                if rep_mode == "bcast":
                    eng.dma_start(dst, _rep3_src(src))
                else:
                    for r in range(L):
                        eng.dma_start(
                            out_rep[m * P : (m + 1) * P, r, cc * N + lo : cc * N + hi],
                            src,
                        )

            # Pass 0 (columns 0:512): k-major, in lockstep with the input
            # stream.  Evictions alternate vector/scalar to clear the
            # end-of-pass burst quickly.
            ps0 = [
                psum_pool.tile([P, N], mybir.dt.float32, name=f"ps0_{m}", tag="ps")
                for m in range(M_T)
            ]
            for k in range(K_T):
                tile_k, off = xwk[k]
                rhs = tile_k[:, off + ROWS_PER_CORE : off + ROWS_PER_CORE + N]
                for m in range(M_T):
                    nc.tensor.matmul(
                        ps0[m][:],
                        tile_k[:, off + m * P : off + (m + 1) * P],
                        rhs,
                        start=(k == 0),
                        stop=(k == K_T - 1),
                    )
            evict0 = [nc.vector.tensor_copy, nc.scalar.copy]
            for m in range(M_T):
                zh = z_pool.tile([P, N], out_mydt, name=f"z0_{m}", tag="z")
                evict0[m % 2](zh[:], ps0[m][:])
                store(zh, m, 0)

            # Pass 1 (columns 512:1024): m-major so each row tile finishes
            # 1.7us after the previous one and its output DMA streams
            # immediately - no end-of-kernel output burst.  The final tile
            # is evicted and stored as two halves on parallel engines to
            # shorten the tail.
            for m in range(M_T):
                ps = psum_pool.tile([P, N], mybir.dt.float32, name=f"ps1_{m}", tag="ps")
                for k in range(K_T):
                    tile_k, off = xwk[k]
                    nc.tensor.matmul(
                        ps[:],
                        tile_k[:, off + m * P : off + (m + 1) * P],
                        wc1[:, k * N : (k + 1) * N],
                        start=(k == 0),
                        stop=(k == K_T - 1),
                    )
                zh = z_pool.tile([P, N], out_mydt, name=f"z1_{m}", tag="z")
                if m == M_T - 1:
                    h = N // 2
                    nc.vector.tensor_copy(zh[:, :h], ps[:, :h])
                    nc.scalar.copy(zh[:, h:], ps[:, h:])
                    store(zh, m, 1, 0, h, eng=nc.sync)
                    store(zh, m, 1, h, N, eng=nc.scalar)
                else:
                    nc.vector.tensor_copy(zh[:], ps[:])
                    store(zh, m, 1)

    _split_multiwaits(nc)
    return nc


_NC_CACHE: dict = {}


def _get_nc(compute_dt: str, out_dt: str, rep_mode: str) -> bass.Bass:
    key = (compute_dt, out_dt, rep_mode)
    if key not in _NC_CACHE:
        _NC_CACHE[key] = _build(compute_dt, out_dt, rep_mode)
    return _NC_CACHE[key]


def kernel(x_q, x_kv, Wq, Wk, Wv, Wproj, _compute_dt=None, _out_dt=None):
    compute_dt = _compute_dt or COMPUTE_DT
    out_dt = _out_dt or OUT_DT
    B, Tkv, C_ = x_kv.shape
    assert (B, Tkv, C_) == (4, 2048, C)

    # Fold the two projections: z = x @ Wv.T @ Wproj.T = x @ WfT
    WfT = (Wv.astype(np.float64).T @ Wproj.astype(np.float64).T).astype(np.float32)

    x_flat = x_kv.reshape(B * Tkv, C)
    in_maps = []
    for c in range(N_CORES):
        shard = x_flat[c * ROWS_PER_CORE : (c + 1) * ROWS_PER_CORE]
        xw = np.concatenate([shard.T, WfT], axis=1)  # [C(k), 2048]
        if compute_dt == "bf16":
            import ml_dtypes

            xw = xw.astype(ml_dtypes.bfloat16)
        else:
            xw = np.ascontiguousarray(xw)
        in_maps.append({"xw": xw})

    nc = _get_nc(compute_dt, out_dt, REP_MODE)
    res = run_bass_kernel_spmd(nc, in_maps, core_ids=list(range(N_CORES)))

    Tq = L * Tkv
    blocks = [res.results[c]["out"] for c in range(N_CORES)]
    out_flat = np.concatenate(blocks, axis=0)  # [B*Tq, C]
    return out_flat.reshape(B, Tq, C).astype(np.float32)


# revision 19
# speedup vs baseline: 1.1026x; 1.1026x over previous
"""Trainium2 Bass kernel for nn_CrossAttention_28183575396415.

The reference block-mask gives every query exactly one key (kv = q_idx // 3),
so the softmax weight is identically 1 and the q/k projections, RMSNorm and
RoPE are dead code.  The module reduces to

    out[b, t] = x_kv[b, t // 3] @ Wv.T @ Wproj.T
              = x_kv[b, t // 3] @ WfT          with WfT = Wv.T @ Wproj.T

Strategy (8 NeuronCores, SPMD):
  - Host folds the two projection matrices into WfT (computed in float64)
    - constant folding of adjacent linear layers.
  - The 4*2048 = 8192 kv rows are row-sharded 8 ways (1024 rows/core).
    Each core's shard is pre-transposed on host so every device DMA is a
    natural contiguous load; the shard and the weight are concatenated into
    one [1024(k), 2048] bf16 input:
        xw[:, :1024]  = x_shard.T   (k on partitions = contraction dim)
        xw[:, 1024:]  = WfT
    The device streams cols 0:1536 (x + first W half - everything pass 0
    needs) as five k-ordered DMAs, and the pass-1-only second W half as one
    trailing DMA, so pass 0's k sems arrive ~25% sooner.
  - Device: z = xT.T @ WfT, K accumulated in PSUM over 8 k-tiles.
    Column half 0 runs k-major (overlapping the input stream), column half 1
    runs m-major so finished row tiles retire one at a time and the output
    DMA stream stays busy until the end instead of bursting at the tail.
  - Each z tile is written to HBM with a single DMA whose source AP repeats
    the tile 3x (stride-0 middle dim) - the t//3 replication - giving this
    core's contiguous [3072, 1024] slice of the flattened output in bf16.
  - Host unshard = concatenate the 8 slices and upcast to float32.
"""

import json
import os

import numpy as np

import concourse.bass as bass
import concourse.mybir as mybir
from bass_rust import AP
from concourse.tile import TileContext
from concourse.vector_clock import ScopedClock
from concourse.bass_utils import run_bass_kernel_spmd

P = 128          # partitions
C = 1024         # model dim
K_T = C // P     # k tiles
M_T = C // P     # row tiles per core shard
N = 512          # matmul free dim (one PSUM bank of fp32)
L = 3            # replication factor (Tq // Tkv)
ROWS_PER_CORE = 1024
N_CORES = 8

# compute dtype: "bf16" (half the input DMA), "f32r"/"f32" for debugging
COMPUTE_DT = os.environ.get("KERNEL_COMPUTE_DT", "bf16")
# output dtype on device: "bf16" (host upcasts) or "f32"
OUT_DT = os.environ.get("KERNEL_OUT_DT", "bf16")
# "bcast": one DMA per z tile with stride-0 replication; "multi": 3 DMAs
REP_MODE = os.environ.get("KERNEL_REP_MODE", "bcast")


class SlimTailTileContext(TileContext):
    """Tile's kernel tail is drain -> barrier -> ~280 serialized per-semaphore
    clear instructions -> barrier (~8 us measured).  The clears only matter if
    the loaded NEFF executes more than once; every kernel() call here builds a
    fresh jit executable (fresh NEFF load, semaphores re-initialized), so skip
    them and the second barrier.  The drain still waits for every DMA queue,
    so outputs are complete before the program ends."""

    def _drain_and_barrier(self, tick_clock, wait_clock):
        drain_inst = self.nc.sync.drain()
        wait_clock.add_sem_waits(
            drain_inst.ins, ScopedClock({None: tick_clock.global_clock})
        )
        popped = self.nc._tile_sem_poison_stack.pop()
        assert popped is self._sem_poison


def _split_multiwaits(nc: bass.Bass) -> None:
    """This container's walrus allows only ONE sync-wait on several
    instruction formats (Drain/CTRL, Matmult's LDWEIGHTS half, ...).  Tile
    can emit more.  Post-pass the serialized BIR: for any instruction with
    >1 on_wait, hoist all but the last wait onto single-wait EventSemaphore
    carriers inserted immediately before it on the same engine (waits then
    execute in queue order - semantics unchanged)."""
    raw = bass.Bass.to_json_bytes(nc)
    j = json.loads(raw)
    for f in j["functions"]:
        for bb in f["blocks"]:
            new_insts = []
            for ins in bb["instructions"]:
                si = ins.get("sync_info")
                waits = si.get("on_wait", []) if si else []
                if len(waits) > 1:
                    for i, w in enumerate(waits[:-1]):
                        carrier = {
                            "engine": ins["engine"],
                            "ins": [],
                            "outs": [],
                            "name": f"{ins['name']}_hw{i}",
                            "opcode": "EventSemaphore",
                            "sync_info": {"on_update": [], "on_wait": [w]},
                        }
                        if "debug" in ins:
                            carrier["debug"] = ins["debug"]
                        new_insts.append(carrier)
                    si["on_wait"] = waits[-1:]
                new_insts.append(ins)
            bb["instructions"] = new_insts
    patched = json.dumps(j).encode()
    nc.to_json_bytes = lambda: patched


def _rep3_src(zh_ap):
    """Source AP reading a [P, N] SBUF tile as [P, L, N] via a stride-0
    middle dim - the DMA replicates each row L times."""
    lay = zh_ap.ap
    assert len(lay) == 2, lay
    return AP(tensor=zh_ap.tensor, offset=zh_ap.offset, ap=[lay[0], [0, L], lay[1]])


def _build(compute_dt: str, out_dt: str, rep_mode: str) -> bass.Bass:
    nc = bass.Bass("TRN2")
    in_mydt = {
        "bf16": mybir.dt.bfloat16,
        "f32r": mybir.dt.float32r,
        "f32": mybir.dt.float32,
    }[compute_dt]
    out_mydt = {"bf16": mybir.dt.bfloat16, "f32": mybir.dt.float32}[out_dt]

    W2 = ROWS_PER_CORE + C  # concatenated [x | w] free dim
    xw = nc.dram_tensor("xw", [C, W2], in_mydt, kind="ExternalInput")
    out = nc.dram_tensor(
        "out", [L * ROWS_PER_CORE, C], out_mydt, kind="ExternalOutput"
    )
    # out row (L*g + r) <- z row g
    out_rep = out.rearrange("(g r) c -> g r c", r=L)  # [1024, L, 1024]

    with SlimTailTileContext(nc) as tc:
        with (
            tc.tile_pool(name="xw", bufs=1) as xw_pool,
            tc.tile_pool(name="psum", bufs=8, space="PSUM") as psum_pool,
            tc.tile_pool(name="zout", bufs=16) as z_pool,
        ):
            # Input: pass 0 only needs the x columns and the first W half
            # (cols 0:1536 of each k-tile row block), so the k-tile DMAs
            # carry just those 3 MiB - each k's sem arrives ~25% sooner -
            # and the second W half (pass-1-only, not needed before ~25us)
            # follows as one final 1 MiB DMA.  First k-tile alone so the PE
            # starts as early as possible; 0.4-0.8 MiB transfers alternate
            # between the two HWDGE rings so k sems pace the PE's
            # cold-then-warm consumption without ever idling it (an idle
            # window would re-throttle the PE clock).
            W1 = ROWS_PER_CORE + N  # x | W-cc0 columns per k
            in_eng = [nc.sync, nc.scalar]
            groups = [[0], [1, 2], [3, 4], [5, 6], [7]]
            xwk = [None] * K_T
            for j, grp in enumerate(groups):
                n = len(grp)
                t = xw_pool.tile([P, n * W1], in_mydt, name=f"xwp{j}", tag=f"xwp{j}")
                src = xw[grp[0] * P : (grp[0] + n) * P, :W1].rearrange(
                    "(g p) m -> p g m", p=P
                )
                dst = t[:].rearrange("p (g m) -> p g m", g=n)
                in_eng[j % 2].dma_start(dst, src)
                for i, k in enumerate(grp):
                    xwk[k] = (t, i * W1)
            wc1 = xw_pool.tile([P, K_T * N], in_mydt, name="wc1", tag="wc1")
            nc.scalar.dma_start(
                wc1[:].rearrange("p (k m) -> p k m", k=K_T),
                xw[:, W1:].rearrange("(k p) m -> p k m", p=P),
            )

            out_eng = [nc.sync, nc.scalar]
            n_trig = [0]

            def store(zh, m, cc, lo=0, hi=N, eng=None):
                dst = out_rep[m * P : (m + 1) * P, :, cc * N + lo : cc * N + hi]
                src = zh[:, lo:hi]
                if eng is None:
                    eng = out_eng[n_trig[0] % 2]
                    n_trig[0] += 1
                if rep_mode == "bcast":
                    eng.dma_start(dst, _rep3_src(src))
                else:
                    for r in range(L):
                        eng.dma_start(
                            out_rep[m * P : (m + 1) * P, r, cc * N + lo : cc * N + hi],
                            src,
                        )

            # Pass 0 (columns 0:512, rows m0-m5 only): k-major, in lockstep
            # with the input stream - 6 matmuls per k-slice (~1.28us warm)
            # matches the ~1.25us-per-k sem cadence, so the PE never idles
            # while the pass ends ~2us sooner than an all-rows pass would.
            # That starts the output stream earlier with a smaller burst, so
            # the output ring backlog fully drains before the final tile.
            # Evictions alternate vector/scalar to clear the burst quickly.
            M_LEAD = 6
            ps0 = [
                psum_pool.tile([P, N], mybir.dt.float32, name=f"ps0_{m}", tag="ps")
                for m in range(M_LEAD)
            ]
            for k in range(K_T):
                tile_k, off = xwk[k]
                rhs = tile_k[:, off + ROWS_PER_CORE : off + ROWS_PER_CORE + N]
                for m in range(M_LEAD):
                    nc.tensor.matmul(
                        ps0[m][:],
                        tile_k[:, off + m * P : off + (m + 1) * P],
                        rhs,
                        start=(k == 0),
                        stop=(k == K_T - 1),
                    )
            evict0 = [nc.vector.tensor_copy, nc.scalar.copy]
            for m in range(M_LEAD):
                zh = z_pool.tile([P, N], out_mydt, name=f"z0_{m}", tag="z")
                evict0[m % 2](zh[:], ps0[m][:])
                store(zh, m, 0)

            # Bridge (columns 0:512, rows m6-m7): m-major on the two PSUM
            # banks the lead pass left free; all k-tiles are resident by now.
            for m in range(M_LEAD, M_T):
                ps = psum_pool.tile([P, N], mybir.dt.float32, name=f"ps0_{m}", tag="ps")
                for k in range(K_T):
                    tile_k, off = xwk[k]
                    nc.tensor.matmul(
                        ps[:],
                        tile_k[:, off + m * P : off + (m + 1) * P],
                        tile_k[:, off + ROWS_PER_CORE : off + ROWS_PER_CORE + N],
                        start=(k == 0),
                        stop=(k == K_T - 1),
                    )
                zh = z_pool.tile([P, N], out_mydt, name=f"z0_{m}", tag="z")
                nc.vector.tensor_copy(zh[:], ps[:])
                store(zh, m, 0)

            # Pass 1 (columns 512:1024): m-major so each row tile finishes
            # 1.7us after the previous one and its output DMA streams
            # immediately - no end-of-kernel output burst.  The final tile
            # is evicted and stored as two halves on parallel engines to
            # shorten the tail.
            for m in range(M_T):
                ps = psum_pool.tile([P, N], mybir.dt.float32, name=f"ps1_{m}", tag="ps")
                for k in range(K_T):
                    tile_k, off = xwk[k]
                    nc.tensor.matmul(
                        ps[:],
                        tile_k[:, off + m * P : off + (m + 1) * P],
                        wc1[:, k * N : (k + 1) * N],
                        start=(k == 0),
                        stop=(k == K_T - 1),
                    )
                zh = z_pool.tile([P, N], out_mydt, name=f"z1_{m}", tag="z")
                if m == M_T - 1:
                    h = N // 2
                    nc.vector.tensor_copy(zh[:, :h], ps[:, :h])
                    nc.scalar.copy(zh[:, h:], ps[:, h:])
                    store(zh, m, 1, 0, h, eng=nc.sync)
                    store(zh, m, 1, h, N, eng=nc.scalar)
                else:
                    nc.vector.tensor_copy(zh[:], ps[:])
                    store(zh, m, 1)

    _split_multiwaits(nc)
    return nc


_NC_CACHE: dict = {}


def _get_nc(compute_dt: str, out_dt: str, rep_mode: str) -> bass.Bass:
    key = (compute_dt, out_dt, rep_mode)
    if key not in _NC_CACHE:
        _NC_CACHE[key] = _build(compute_dt, out_dt, rep_mode)
    return _NC_CACHE[key]


def kernel(x_q, x_kv, Wq, Wk, Wv, Wproj, _compute_dt=None, _out_dt=None):
    compute_dt = _compute_dt or COMPUTE_DT
    out_dt = _out_dt or OUT_DT
    B, Tkv, C_ = x_kv.shape
    assert (B, Tkv, C_) == (4, 2048, C)

    # Fold the two projections: z = x @ Wv.T @ Wproj.T = x @ WfT
    WfT = (Wv.astype(np.float64).T @ Wproj.astype(np.float64).T).astype(np.float32)

    x_flat = x_kv.reshape(B * Tkv, C)
    in_maps = []
    for c in range(N_CORES):
        shard = x_flat[c * ROWS_PER_CORE : (c + 1) * ROWS_PER_CORE]
        xw = np.concatenate([shard.T, WfT], axis=1)  # [C(k), 2048]
        if compute_dt == "bf16":
            import ml_dtypes

            xw = xw.astype(ml_dtypes.bfloat16)
        else:
            xw = np.ascontiguousarray(xw)
        in_maps.append({"xw": xw})

    nc = _get_nc(compute_dt, out_dt, REP_MODE)
    res = run_bass_kernel_spmd(nc, in_maps, core_ids=list(range(N_CORES)))

    Tq = L * Tkv
    blocks = [res.results[c]["out"] for c in range(N_CORES)]
    out_flat = np.concatenate(blocks, axis=0)  # [B*Tq, C]
    return out_flat.reshape(B, Tq, C).astype(np.float32)
